# revision 1
# baseline (speedup 1.0000x reference)
"""Trainium2 Bass kernel for nn_Conv1d_fft (B=16, Cin=Cout=128, L=4096, K=129, PAD=32).

The reference computes the conv via FFT with circular length 4160, output
truncated to 4032. Because 4160 >= L + 2*PAD and only the first 4032 samples
are kept, the circular wrap only ever touches zero padding, so the whole op
is exactly a plain cross-correlation (PyTorch-style Conv1d with padding=32)
plus a bias:

    out[b, o, n] = bias[o] + sum_{i, t} weight[o, i, t] * xp[b, i, n + t]

with xp = x zero-padded by 32 on each side (length 4160), n in [0, 4032).

Strategy: data-parallel over batch (2 batches per core, 8 cores). Per core,
the conv is 129 shifted matmuls accumulated in PSUM per output tile:
lhsT = weight[:, :, t] transposed to (Cin, Cout), rhs = xp window (Cin, 504).
Weight is pre-transposed on the host to (Cin, K*Cout) so DMA is contiguous.
"""

import os
import numpy as np

import concourse.bass as bass
import concourse.bacc as bacc
import concourse.tile as tile
import concourse.mybir as mybir
from concourse.bass_utils import run_bass_kernel_spmd

B, CIN, COUT, L, K = 16, 128, 128, 4096, 129
PAD = 32
OUT_LEN = 2 * PAD + L - (K - 1)  # 4032
LP = L + 2 * PAD                 # 4160
N_CORES = 8
BPC = B // N_CORES               # batches per core
TW = 504                         # output tile width (8 * 504 = 4032)
NT = OUT_LEN // TW

F32 = mybir.dt.float32
BF16 = mybir.dt.bfloat16
F32R = mybir.dt.float32r

_cache = {}


def _build_program(mode: str, order: str):
    """mode: f32 | f32r | bf16;  order: tap_inner | tap_outer."""
    io_dt = BF16 if mode == "bf16" else F32
    nc = bacc.Bacc("TRN2", target_bir_lowering=False, debug=False,
                   num_devices=N_CORES)

    x_d = nc.dram_tensor("x", [BPC, CIN, LP], io_dt, kind="ExternalInput").ap()
    w_d = nc.dram_tensor("w", [CIN, K * COUT], io_dt, kind="ExternalInput").ap()
    b_d = nc.dram_tensor("b", [COUT, 1], F32, kind="ExternalInput").ap()
    o_d = nc.dram_tensor("out", [BPC, COUT, OUT_LEN], F32,
                         kind="ExternalOutput").ap()

    def mm_ap(ap):
        return ap.bitcast(F32R) if mode == "f32r" else ap

    with tile.TileContext(nc) as tc:
        with (
            tc.tile_pool(name="wp", bufs=1) as wp,
            tc.tile_pool(name="xp", bufs=1) as xp,
            tc.tile_pool(name="bp", bufs=1) as bp,
            tc.tile_pool(name="op", bufs=4) as op,
            tc.tile_pool(name="ps", bufs=8, space=bass.MemorySpace.PSUM) as ps,
        ):
            w_sb = wp.tile([CIN, K * COUT], io_dt)
            nc.sync.dma_start(w_sb[:], w_d[:])
            b_sb = bp.tile([COUT, 1], F32)
            nc.sync.dma_start(b_sb[:], b_d[:])
            x_sb = []
            for b in range(BPC):
                t_ = xp.tile([CIN, LP], io_dt, tag=f"x{b}")
                nc.sync.dma_start(t_[:], x_d[b])
                x_sb.append(t_)

            def drain(psum_tile, b, j):
                o_sb = op.tile([COUT, TW], F32)
                nc.vector.tensor_scalar_add(o_sb[:], psum_tile[:], b_sb[:])
                nc.sync.dma_start(o_d[b][:, j * TW:(j + 1) * TW], o_sb[:])

            if order == "tap_inner":
                for b in range(BPC):
                    for j in range(NT):
                        acc = ps.tile([COUT, TW], F32)
                        for t in range(K):
                            nc.tensor.matmul(
                                acc[:],
                                mm_ap(w_sb[:, t * COUT:(t + 1) * COUT]),
                                mm_ap(x_sb[b][:, j * TW + t: j * TW + t + TW]),
                                start=(t == 0), stop=(t == K - 1),
                            )
                        drain(acc, b, j)
            else:  # tap_outer
                for b in range(BPC):
                    accs = [ps.tile([COUT, TW], F32, tag=f"acc{j}")
                            for j in range(NT)]
                    for t in range(K):
                        for j in range(NT):
                            nc.tensor.matmul(
                                accs[j][:],
                                mm_ap(w_sb[:, t * COUT:(t + 1) * COUT]),
                                mm_ap(x_sb[b][:, j * TW + t: j * TW + t + TW]),
                                start=(t == 0), stop=(t == K - 1),
                            )
                    for j in range(NT):
                        drain(accs[j], b, j)

    nc.compile()
    return nc


def _get_program(mode, order):
    key = (mode, order)
    if key not in _cache:
        _cache[key] = _build_program(mode, order)
    return _cache[key]


def kernel(x, weight, bias, _trace=False, _trace_kwargs=None):
    mode = os.environ.get("BASS_CONV_MODE", "f32")
    order = os.environ.get("BASS_CONV_ORDER", "tap_inner")
    nc = _get_program(mode, order)

    io_np = np.dtype("bfloat16") if mode == "bf16" else np.float32
    if mode == "bf16":
        import ml_dtypes  # noqa: F401  (registers bfloat16 with numpy)
        io_np = ml_dtypes.bfloat16

    xp = np.zeros((B, CIN, LP), dtype=np.float32)
    xp[:, :, PAD:PAD + L] = x
    xp = np.ascontiguousarray(xp.astype(io_np))
    # (Cout, Cin, K) -> (Cin, K, Cout) so per-tap lhsT slices are contiguous
    wT = np.ascontiguousarray(
        np.transpose(weight, (1, 2, 0)).astype(io_np)).reshape(CIN, K * COUT)
    b2 = np.ascontiguousarray(bias.astype(np.float32).reshape(COUT, 1))

    in_maps = [
        {"x": xp[c * BPC:(c + 1) * BPC], "w": wT, "b": b2}
        for c in range(N_CORES)
    ]
    res = run_bass_kernel_spmd(
        nc, in_maps, list(range(N_CORES)),
        trace=_trace, **(_trace_kwargs or {}),
    )
    out = np.concatenate([res.results[c]["out"] for c in range(N_CORES)],
                         axis=0).astype(np.float32)
    if _trace:
        return out, res
    return out
